# revision 22
# baseline (speedup 1.0000x reference)
"""MoE experts FFN kernel for Trainium2 (8 NeuronCores, expert parallel).

Reference computation (per expert e of 8):
    inter = hidden_states[e] @ gate_up_w[e]        # (C,H)@(H,2I) -> (C,2I)
    gate, up = split(inter, 2, axis=-1)
    act = silu(gate) * up                          # (C,I)
    out[e] = act @ down_w[e]                       # (C,I)@(I,H) -> (C,H)

E == n_cores == 8, so each core owns one expert end-to-end (no collectives).

Device-side layout trick: the PE computes lhsT.T @ rhs with the contraction
dim on partitions for BOTH operands.  Feeding x transposed (Xt = x.T, [H,C])
lets every matmul use naturally-laid-out weights as the stationary operand
and produces transposed intermediates:

    interT[f, c] = sum_h Wgu[h, f] * Xt[h, c]      (lhsT = Wgu tile, rhs = Xt)
    actT          = silu(gateT) * upT              (elementwise, any layout)
    outT[h, c]   = sum_i Wd[i, h] * actT[i, c]     (lhsT = Wd tile, rhs = actT)

The host transposes x on the way in and outT on the way out, casts inputs to
bf16 (fp32 accumulate in PSUM keeps the contraction accurate), and pre-packs
the weights into per-output-block tiles -- [block, p, kt, f] -- so every
weight-group load is one fully-contiguous DMA with 4KB-per-partition runs.
"""

from contextlib import ExitStack

import numpy as np
import ml_dtypes

E, C, H, I = 8, 2048, 2048, 2816
F2 = 2 * I          # fused gate+up columns
P = 128             # partitions
NF = 512            # matmul moving free dim == one PSUM bank of fp32
KT = H // P         # 16 k-tiles over H (matmul 1)
IT = I // P         # 22 i-tiles over I
HT = H // P         # 16 h-tiles over H (matmul 2 output)
FT = F2 // P        # 44 f-blocks (22 gate + 22 up)
CT = C // NF        # 4 c-chunks

_NC_CACHE = {}


def _build_nc(compute="bfloat16"):
    if compute in _NC_CACHE:
        return _NC_CACHE[compute]

    import concourse.bacc as bacc
    import concourse.tile as tile
    from concourse import mybir

    cdt = getattr(mybir.dt, compute)
    f32 = mybir.dt.float32
    AFT = mybir.ActivationFunctionType

    nc = bacc.Bacc(None, target_bir_lowering=False, name="moe_expert_ffn")

    # wgu/wd arrive pre-packed: [block, p, kt* , f] with (kt, f) contiguous
    # per partition p (see make_in_maps).
    xt_d = nc.dram_tensor("xt", [H, C], cdt, kind="ExternalInput")
    wgu_d = nc.dram_tensor("wgu", [FT, P, KT, P], cdt, kind="ExternalInput")
    wd_d = nc.dram_tensor("wd", [HT, P, IT, P], cdt, kind="ExternalInput")
    outT_d = nc.dram_tensor("outT", [H, C], f32, kind="ExternalOutput")

    xt_r = xt_d.ap().rearrange("(kt p) c -> p kt c", p=P)       # [128, KT, C]
    wgu_a = wgu_d.ap()
    wd_a = wd_d.ap()
    outT_a = outT_d.ap()

    with tile.TileContext(nc) as tc, ExitStack() as ctx:
        singles = ctx.enter_context(tc.tile_pool(name="singles", bufs=1))
        wpool = ctx.enter_context(tc.tile_pool(name="wpool", bufs=2))
        tpool = ctx.enter_context(tc.tile_pool(name="tpool", bufs=2))
        opool = ctx.enter_context(tc.tile_pool(name="opool", bufs=3))
        psum = ctx.enter_context(tc.tile_pool(name="psum", bufs=4, space="PSUM"))

        def glu(i, c, g_ps, u_ps):
            c0 = c * NF
            s_sb = tpool.tile([P, NF], f32, tag="sig", name=f"sig{i}_{c}")
            nc.scalar.activation(out=s_sb, in_=g_ps, func=AFT.Sigmoid)
            nc.vector.tensor_mul(s_sb, g_ps, s_sb)
            nc.vector.tensor_mul(act_sb[i][:, c0 : c0 + NF], s_sb, u_ps)

        def load_gu_weights(i, bufs=3):
            wg = wpool.tile(
                [P, KT, P], cdt, tag="wg", name=f"wg{i}", bufs=bufs
            )
            wu = wpool.tile(
                [P, KT, P], cdt, tag="wu", name=f"wu{i}", bufs=bufs
            )
            nc.sync.dma_start(out=wg, in_=wgu_a[i])
            nc.sync.dma_start(out=wu, in_=wgu_a[IT + i])
            return wg, wu

        # Dummy matmuls on zeroed tiles fill the PE's dead window while the
        # first inputs stream in: HAM un-throttles (1.2 -> 2.4 GHz) after
        # ~3.4us of sustained activity, so the real matmuls start warm
        # instead of paying the cold-clock ramp.  Sized to end just before
        # data arrives (~14us) so they never delay real work, with the gap
        # kept under the ~3.4us re-throttle window.
        wz = singles.tile([P, P], cdt, tag="wz", name="wz")
        xz = singles.tile([P, NF], cdt, tag="xz", name="xz")
        nc.vector.memset(wz, 0.0)
        nc.vector.memset(xz, 0.0)
        warm_ps = psum.tile([P, NF], f32, tag="gps", name="warm_ps")
        for w in range(24):
            nc.tensor.matmul(warm_ps, wz, xz, start=True, stop=True)

        # kt=0 of Xt lands as four small chunk tiles, DMA'd before everything
        # else, so the first matmuls fire during NEFF bring-up instead of
        # behind the whole 8.4MB Xt load.  i=0's weights go next.
        xt0_sb = []
        for c in range(CT):
            t = singles.tile([P, NF], cdt, tag=f"xt0_{c}", name=f"xt0_{c}")
            nc.sync.dma_start(out=t, in_=xt_r[:, 0, c * NF : (c + 1) * NF])
            xt0_sb.append(t)

        wg0, wu0 = load_gu_weights(0)

        # Rest of Xt resident in SBUF, one tile per k-tile so DMAs/readers
        # don't false-share dependency state.
        xt_sb = [None]
        for kt in range(1, KT):
            t = singles.tile([P, C], cdt, tag=f"xt{kt}", name=f"xt{kt}")
            nc.sync.dma_start(out=t, in_=xt_r[:, kt, :])
            xt_sb.append(t)

        def xt_ap(kt, c):
            if kt == 0:
                return xt0_sb[c][:, :]
            return xt_sb[kt][:, c * NF : (c + 1) * NF]

        # actT resident in SBUF, one tile per i-tile.
        act_sb = [
            singles.tile([P, C], cdt, tag=f"act{i}", name=f"act{i}")
            for i in range(IT)
        ]

        # ---- phase 2: interT = Wgu.T @ Xt, actT = silu(gateT)*upT ----
        # i=0 runs kt-outer across all 8 PSUM banks: each matmul depends on
        # one xt k-tile only, so compute overlaps the initial Xt load.
        g0_ps = [
            psum.tile([P, NF], f32, tag="gps", name=f"gps0_{c}") for c in range(CT)
        ]
        u0_ps = [
            psum.tile([P, NF], f32, tag="ups", name=f"ups0_{c}") for c in range(CT)
        ]
        for kt in range(KT):
            for w_t, ps in ((wg0, g0_ps), (wu0, u0_ps)):
                for c in range(CT):
                    c0 = c * NF
                    nc.tensor.matmul(
                        ps[c],
                        w_t[:, kt, :],
                        xt_ap(kt, c),
                        start=(kt == 0),
                        stop=(kt == KT - 1),
                    )
        for c in range(CT):
            glu(0, c, g0_ps[c], u0_ps[c])

        for i in range(1, IT):
            wg, wu = load_gu_weights(i)
            for c in range(CT):
                c0 = c * NF
                g_ps = psum.tile([P, NF], f32, tag="gps", name=f"gps{i}_{c}")
                u_ps = psum.tile([P, NF], f32, tag="ups", name=f"ups{i}_{c}")
                for kt in range(KT):
                    nc.tensor.matmul(
                        g_ps,
                        wg[:, kt, :],
                        xt_ap(kt, c),
                        start=(kt == 0),
                        stop=(kt == KT - 1),
                    )
                for kt in range(KT):
                    nc.tensor.matmul(
                        u_ps,
                        wu[:, kt, :],
                        xt_ap(kt, c),
                        start=(kt == 0),
                        stop=(kt == KT - 1),
                    )
                glu(i, c, g_ps, u_ps)

        # ---- phase 3: outT = Wd.T @ actT ----
        for h in range(HT):
            wd_t = wpool.tile([P, IT, P], cdt, tag="wd", name=f"wd{h}")
            h0 = h * P
            nc.sync.dma_start(out=wd_t, in_=wd_a[h])
            for c in range(CT):
                c0 = c * NF
                # reuse phase-2 bank groups (8 banks total; no room for a
                # third tag)
                o_ps = psum.tile(
                    [P, NF], f32, tag="gps" if c % 2 == 0 else "ups",
                    name=f"ops{h}_{c}",
                )
                for it in range(IT):
                    nc.tensor.matmul(
                        o_ps,
                        wd_t[:, it, :],
                        act_sb[it][:, c0 : c0 + NF],
                        start=(it == 0),
                        stop=(it == IT - 1),
                    )
                o_sb = opool.tile([P, NF], f32, tag="osb", name=f"osb{h}_{c}")
                nc.vector.tensor_copy(out=o_sb, in_=o_ps)
                nc.sync.dma_start(
                    out=outT_a[h0 : h0 + P, c0 : c0 + NF], in_=o_sb
                )

    nc.compile()
    _NC_CACHE[compute] = nc
    return nc


def _np_dtype(compute):
    return {"bfloat16": ml_dtypes.bfloat16, "float16": np.float16, "float32r": np.float32}[compute]


def _pack_w(w, n_k, n_b):
    """[K, B*P] -> [B, P, n_k, P] with (kt, f) contiguous per partition p."""
    return np.ascontiguousarray(
        w.reshape(n_k, P, n_b, P).transpose(2, 1, 0, 3)
    )


def make_in_maps(hidden_states, gate_up_w, down_w, compute="bfloat16"):
    dt = _np_dtype(compute)
    in_maps = []
    for e in range(E):
        in_maps.append(
            {
                "xt": np.ascontiguousarray(hidden_states[e].T).astype(dt),
                "wgu": _pack_w(gate_up_w[e].astype(dt), KT, FT),
                "wd": _pack_w(down_w[e].astype(dt), IT, HT),
            }
        )
    return in_maps


def run_hw(in_maps, compute="bfloat16", trace=False, **kwargs):
    from concourse import bass_utils

    if trace:
        # local-only devloop: skip the artifact-bucket upload
        bass_utils.upload_artifacts = lambda tmpdir: f"local:{tmpdir}"
    nc = _build_nc(compute)
    return bass_utils.run_bass_kernel_spmd(
        nc, in_maps, core_ids=list(range(E)), trace=trace, **kwargs
    )


def kernel(hidden_states, gate_up_w, down_w):
    compute = "float16"
    hidden_states = np.asarray(hidden_states)
    gate_up_w = np.asarray(gate_up_w)
    down_w = np.asarray(down_w)
    in_maps = make_in_maps(hidden_states, gate_up_w, down_w, compute)
    res = run_hw(in_maps, compute)
    out = np.empty((E, C, H), dtype=np.float32)
    for e in range(E):
        out[e] = res.results[e]["outT"].T
    return out


# revision 23
# speedup vs baseline: 1.0011x; 1.0011x over previous
"""MoE experts FFN kernel for Trainium2 (8 NeuronCores, expert parallel).

Reference computation (per expert e of 8):
    inter = hidden_states[e] @ gate_up_w[e]        # (C,H)@(H,2I) -> (C,2I)
    gate, up = split(inter, 2, axis=-1)
    act = silu(gate) * up                          # (C,I)
    out[e] = act @ down_w[e]                       # (C,I)@(I,H) -> (C,H)

E == n_cores == 8, so each core owns one expert end-to-end (no collectives).

Device-side layout trick: the PE computes lhsT.T @ rhs with the contraction
dim on partitions for BOTH operands.  Feeding x transposed (Xt = x.T, [H,C])
lets every matmul use naturally-laid-out weights as the stationary operand
and produces transposed intermediates:

    interT[f, c] = sum_h Wgu[h, f] * Xt[h, c]      (lhsT = Wgu tile, rhs = Xt)
    actT          = silu(gateT) * upT              (elementwise, any layout)
    outT[h, c]   = sum_i Wd[i, h] * actT[i, c]     (lhsT = Wd tile, rhs = actT)

The host transposes x on the way in and outT on the way out, casts inputs to
bf16 (fp32 accumulate in PSUM keeps the contraction accurate), and pre-packs
the weights into per-output-block tiles -- [block, p, kt, f] -- so every
weight-group load is one fully-contiguous DMA with 4KB-per-partition runs.
"""

from contextlib import ExitStack

import numpy as np
import ml_dtypes

E, C, H, I = 8, 2048, 2048, 2816
F2 = 2 * I          # fused gate+up columns
P = 128             # partitions
NF = 512            # matmul moving free dim == one PSUM bank of fp32
KT = H // P         # 16 k-tiles over H (matmul 1)
IT = I // P         # 22 i-tiles over I
HT = H // P         # 16 h-tiles over H (matmul 2 output)
FT = F2 // P        # 44 f-blocks (22 gate + 22 up)
CT = C // NF        # 4 c-chunks

_NC_CACHE = {}


def _build_nc(compute="bfloat16"):
    if compute in _NC_CACHE:
        return _NC_CACHE[compute]

    import concourse.bacc as bacc
    import concourse.tile as tile
    from concourse import mybir

    cdt = getattr(mybir.dt, compute)
    f32 = mybir.dt.float32
    AFT = mybir.ActivationFunctionType

    nc = bacc.Bacc(None, target_bir_lowering=False, name="moe_expert_ffn")

    # wgu/wd arrive pre-packed: [block, p, kt* , f] with (kt, f) contiguous
    # per partition p (see make_in_maps).
    xt_d = nc.dram_tensor("xt", [H, C], cdt, kind="ExternalInput")
    wgu_d = nc.dram_tensor("wgu", [FT, P, KT, P], cdt, kind="ExternalInput")
    wd_d = nc.dram_tensor("wd", [HT, P, IT, P], cdt, kind="ExternalInput")
    odt = cdt if compute == "float16" else f32
    outT_d = nc.dram_tensor("outT", [H, C], odt, kind="ExternalOutput")

    xt_r = xt_d.ap().rearrange("(kt p) c -> p kt c", p=P)       # [128, KT, C]
    wgu_a = wgu_d.ap()
    wd_a = wd_d.ap()
    outT_a = outT_d.ap()

    with tile.TileContext(nc) as tc, ExitStack() as ctx:
        singles = ctx.enter_context(tc.tile_pool(name="singles", bufs=1))
        wpool = ctx.enter_context(tc.tile_pool(name="wpool", bufs=2))
        tpool = ctx.enter_context(tc.tile_pool(name="tpool", bufs=2))
        opool = ctx.enter_context(tc.tile_pool(name="opool", bufs=3))
        psum = ctx.enter_context(tc.tile_pool(name="psum", bufs=4, space="PSUM"))

        def glu(i, c, g_ps, u_ps):
            c0 = c * NF
            s_sb = tpool.tile([P, NF], f32, tag="sig", name=f"sig{i}_{c}")
            nc.scalar.activation(out=s_sb, in_=g_ps, func=AFT.Sigmoid)
            nc.vector.tensor_mul(s_sb, g_ps, s_sb)
            nc.vector.tensor_mul(act_sb[i][:, c0 : c0 + NF], s_sb, u_ps)

        def load_gu_weights(i, bufs=3):
            wg = wpool.tile(
                [P, KT, P], cdt, tag="wg", name=f"wg{i}", bufs=bufs
            )
            wu = wpool.tile(
                [P, KT, P], cdt, tag="wu", name=f"wu{i}", bufs=bufs
            )
            nc.sync.dma_start(out=wg, in_=wgu_a[i])
            nc.sync.dma_start(out=wu, in_=wgu_a[IT + i])
            return wg, wu

        # Dummy matmuls on zeroed tiles fill the PE's dead window while the
        # first inputs stream in: HAM un-throttles (1.2 -> 2.4 GHz) after
        # ~3.4us of sustained activity, so the real matmuls start warm
        # instead of paying the cold-clock ramp.  Sized to end just before
        # data arrives (~14us) so they never delay real work, with the gap
        # kept under the ~3.4us re-throttle window.
        wz = singles.tile([P, P], cdt, tag="wz", name="wz")
        xz = singles.tile([P, NF], cdt, tag="xz", name="xz")
        nc.vector.memset(wz, 0.0)
        nc.vector.memset(xz, 0.0)
        warm_ps = psum.tile([P, NF], f32, tag="gps", name="warm_ps")
        for w in range(24):
            nc.tensor.matmul(warm_ps, wz, xz, start=True, stop=True)

        # kt=0 of Xt lands as four small chunk tiles, DMA'd before everything
        # else, so the first matmuls fire during NEFF bring-up instead of
        # behind the whole 8.4MB Xt load.  i=0's weights go next.
        xt0_sb = []
        for c in range(CT):
            t = singles.tile([P, NF], cdt, tag=f"xt0_{c}", name=f"xt0_{c}")
            nc.sync.dma_start(out=t, in_=xt_r[:, 0, c * NF : (c + 1) * NF])
            xt0_sb.append(t)

        wg0, wu0 = load_gu_weights(0)

        # Rest of Xt resident in SBUF, one tile per k-tile so DMAs/readers
        # don't false-share dependency state.
        xt_sb = [None]
        for kt in range(1, KT):
            t = singles.tile([P, C], cdt, tag=f"xt{kt}", name=f"xt{kt}")
            nc.sync.dma_start(out=t, in_=xt_r[:, kt, :])
            xt_sb.append(t)

        def xt_ap(kt, c):
            if kt == 0:
                return xt0_sb[c][:, :]
            return xt_sb[kt][:, c * NF : (c + 1) * NF]

        # actT resident in SBUF, one tile per i-tile.
        act_sb = [
            singles.tile([P, C], cdt, tag=f"act{i}", name=f"act{i}")
            for i in range(IT)
        ]

        # ---- phase 2: interT = Wgu.T @ Xt, actT = silu(gateT)*upT ----
        # i=0 runs kt-outer across all 8 PSUM banks: each matmul depends on
        # one xt k-tile only, so compute overlaps the initial Xt load.
        g0_ps = [
            psum.tile([P, NF], f32, tag="gps", name=f"gps0_{c}") for c in range(CT)
        ]
        u0_ps = [
            psum.tile([P, NF], f32, tag="ups", name=f"ups0_{c}") for c in range(CT)
        ]
        for kt in range(KT):
            for w_t, ps in ((wg0, g0_ps), (wu0, u0_ps)):
                for c in range(CT):
                    c0 = c * NF
                    nc.tensor.matmul(
                        ps[c],
                        w_t[:, kt, :],
                        xt_ap(kt, c),
                        start=(kt == 0),
                        stop=(kt == KT - 1),
                    )
        for c in range(CT):
            glu(0, c, g0_ps[c], u0_ps[c])

        for i in range(1, IT):
            wg, wu = load_gu_weights(i)
            for c in range(CT):
                c0 = c * NF
                g_ps = psum.tile([P, NF], f32, tag="gps", name=f"gps{i}_{c}")
                u_ps = psum.tile([P, NF], f32, tag="ups", name=f"ups{i}_{c}")
                for kt in range(KT):
                    nc.tensor.matmul(
                        g_ps,
                        wg[:, kt, :],
                        xt_ap(kt, c),
                        start=(kt == 0),
                        stop=(kt == KT - 1),
                    )
                for kt in range(KT):
                    nc.tensor.matmul(
                        u_ps,
                        wu[:, kt, :],
                        xt_ap(kt, c),
                        start=(kt == 0),
                        stop=(kt == KT - 1),
                    )
                glu(i, c, g_ps, u_ps)

        # ---- phase 3: outT = Wd.T @ actT ----
        for h in range(HT):
            wd_t = wpool.tile([P, IT, P], cdt, tag="wd", name=f"wd{h}")
            h0 = h * P
            nc.sync.dma_start(out=wd_t, in_=wd_a[h])
            for c in range(CT):
                c0 = c * NF
                # reuse phase-2 bank groups (8 banks total; no room for a
                # third tag)
                o_ps = psum.tile(
                    [P, NF], f32, tag="gps" if c % 2 == 0 else "ups",
                    name=f"ops{h}_{c}",
                )
                for it in range(IT):
                    nc.tensor.matmul(
                        o_ps,
                        wd_t[:, it, :],
                        act_sb[it][:, c0 : c0 + NF],
                        start=(it == 0),
                        stop=(it == IT - 1),
                    )
                o_sb = opool.tile([P, NF], odt, tag="osb", name=f"osb{h}_{c}")
                nc.vector.tensor_copy(out=o_sb, in_=o_ps)
                nc.sync.dma_start(
                    out=outT_a[h0 : h0 + P, c0 : c0 + NF], in_=o_sb
                )

    nc.compile()
    _NC_CACHE[compute] = nc
    return nc


def _np_dtype(compute):
    return {"bfloat16": ml_dtypes.bfloat16, "float16": np.float16, "float32r": np.float32}[compute]


def _pack_w(w, n_k, n_b):
    """[K, B*P] -> [B, P, n_k, P] with (kt, f) contiguous per partition p."""
    return np.ascontiguousarray(
        w.reshape(n_k, P, n_b, P).transpose(2, 1, 0, 3)
    )


def make_in_maps(hidden_states, gate_up_w, down_w, compute="bfloat16"):
    dt = _np_dtype(compute)
    in_maps = []
    for e in range(E):
        in_maps.append(
            {
                "xt": np.ascontiguousarray(hidden_states[e].T).astype(dt),
                "wgu": _pack_w(gate_up_w[e].astype(dt), KT, FT),
                "wd": _pack_w(down_w[e].astype(dt), IT, HT),
            }
        )
    return in_maps


def run_hw(in_maps, compute="bfloat16", trace=False, **kwargs):
    from concourse import bass_utils

    if trace:
        # local-only devloop: skip the artifact-bucket upload
        bass_utils.upload_artifacts = lambda tmpdir: f"local:{tmpdir}"
    nc = _build_nc(compute)
    return bass_utils.run_bass_kernel_spmd(
        nc, in_maps, core_ids=list(range(E)), trace=trace, **kwargs
    )


def kernel(hidden_states, gate_up_w, down_w):
    compute = "float16"
    hidden_states = np.asarray(hidden_states)
    gate_up_w = np.asarray(gate_up_w)
    down_w = np.asarray(down_w)
    in_maps = make_in_maps(hidden_states, gate_up_w, down_w, compute)
    res = run_hw(in_maps, compute)
    out = np.empty((E, C, H), dtype=np.float32)
    for e in range(E):
        out[e] = res.results[e]["outT"].T
    return out
